# revision 12
# baseline (speedup 1.0000x reference)
"""Trainium2 Bass kernel for the AdaptiveSNN problem.

Strategy (data parallel: batch 16384 -> 2048/core across 8 NeuronCores):
  - Block-streamed start: the 2048-col batch is split into 3 column blocks
    (512/768/768). Each block's xt DMA, 7 fp32 K-tile matmuls
    (cur1 = W1 @ x^T + b1) and 25-step LIF chain are pipelined
    independently, so the first LIF starts ~11us instead of waiting ~45us
    for the full cur1 (DMA of x is 6.4MB/core ~18us).
  - Layer-1 LIF is one fused custom DVE op per (step, block); the DVE runs
    at 1 elem/lane/cycle @0.96GHz, so ~2.3us/step is its floor and it is
    the pacing engine. Spikes are Sign(m-1) (+-1 bf16) on the Scalar
    engine; layer-2 chunk matmuls use 0.5*W2 hi/lo bf16 moving plus a K=1
    constant-row matmul (exact to ~1e-7).
  - Layer-2 LIF ([128,160]) is one lagged custom DVE op per step reading
    cur2 from PSUM directly; only mem2 is DMA'd out and the host
    recomputes spk2 = (mem2 > 1) bit-exactly.
"""
import numpy as np
import ml_dtypes

import concourse.bacc as bacc
import concourse.mybir as mybir
import concourse.tile as tile
from concourse.tile import add_dep_helper
import concourse.dve_ops as dve_ops
from concourse.dve_spec import Spec, Src0, Src1, C0, C1
from concourse.dve_ops import DveOp
from concourse.bass_utils import run_bass_kernel_spmd

F32 = mybir.dt.float32
BF16 = mybir.dt.bfloat16
Alu = mybir.AluOpType

N_CORES = 8
B_FULL = 16384
B = B_FULL // N_CORES          # 2048 batch rows per core
D_IN = 784                     # 28*28
H1 = 128
H2 = 10
STEPS = 25
KT = 112                       # K-tile size: 784 = 7 * 112
NKT = D_IN // KT
NCHUNK = B // 128              # 16 batch chunks per core
THRESH = 1.0

# Column blocks (name, col0, width) — all on the DVE.
BLOCKS = [
    ("A", 0,    512),
    ("B", 512,  768),
    ("C", 1280, 768),
]
# emission-wave delay per block; chunk matmuls at wave t+CHUNK_WAVE
DELAY = {"A": 0, "B": 3, "C": 5}
CHUNK_WAVE = {"A": 1, "B": 3, "C": 5}
LAG = 6          # lif2(s) on the DVE at wave s+LAG
N_WAVES = STEPS + LAG + 1
COPY_WAVE = {"B": 2, "C": 4}   # cur1 psum->sbuf copies for later blocks


def _register_lif():
    """Custom DVE op: out = ((in0*s0 + in1) - (in0 > s1))."""
    if "LIF_STEP_ANT" in dve_ops._SUB_OPCODE_FOR_NAME:
        return next(op for op in dve_ops.OPS if op.name == "LIF_STEP_ANT")
    op = DveOp(
        "LIF_STEP_ANT",
        Spec(
            body=(Src0 * C0 + Src1) - (Src0 > C1),
            reference=lambda in0, in1, s0, s1, imm2: (
                (in0 * s0 + in1) - (in0 > s1).astype(np.float32)
            ),
        ),
        subdim=False,
        uops_sha={"v3": "4d971942aba05d49", "v4": "da6677450a1cb1b9"},
    )
    dve_ops.OPS.append(op)
    dve_ops._SUB_OPCODE_FOR_NAME[op.name] = (
        dve_ops._CUSTOM_DVE_ROW_BASE + len(dve_ops.OPS) - 1
    )
    dve_ops.CUSTOM_DVE_SPECS[op.name] = op.spec
    return op


_GRAPH_CACHE = {}


def _build_graph(beta1: float, beta2: float):
    key = (beta1, beta2)
    if key in _GRAPH_CACHE:
        return _GRAPH_CACHE[key]
    LIF = _register_lif()
    Sign = mybir.ActivationFunctionType.Sign
    Ident = mybir.ActivationFunctionType.Identity

    nc = bacc.Bacc("TRN2", target_bir_lowering=False, debug=False,
                   num_devices=N_CORES)

    xt_d = nc.dram_tensor("xt", [NKT, KT, B], F32, kind="ExternalInput").ap()
    w1t_d = nc.dram_tensor("w1t", [KT, NKT * H1], F32, kind="ExternalInput").ap()
    b1_d = nc.dram_tensor("b1", [H1, 1], F32, kind="ExternalInput").ap()
    w2h_d = nc.dram_tensor("w2h", [H1, H2], BF16, kind="ExternalInput").ap()
    w2l_d = nc.dram_tensor("w2l", [H1, H2], BF16, kind="ExternalInput").ap()
    cc_d = nc.dram_tensor("cc160", [1, NCHUNK * H2], F32, kind="ExternalInput").ap()

    out_mem = nc.dram_tensor("out_mem", [STEPS, 128, NCHUNK * H2], F32,
                             kind="ExternalOutput").ap()

    with tile.TileContext(nc) as tc:
        with tc.tile_pool(name="const", bufs=1) as cpool, \
             tc.tile_pool(name="xin", bufs=1) as xpool, \
             tc.tile_pool(name="m1p", bufs=3) as m1pool, \
             tc.tile_pool(name="m2p", bufs=4) as m2pool, \
             tc.tile_pool(name="u2p", bufs=2) as u2pool, \
             tc.tile_pool(name="sgp", bufs=3) as sgpool, \
             tc.tile_pool(name="ps", bufs=8, space="PSUM") as pspool:

            # preload ACT tables (Sign + Identity) before anything else
            warm_t = cpool.tile([H1, 1], F32, tag="warm")
            nc.scalar.activation(warm_t[:], nc.const_aps.tensor(0.0, (H1, 1)),
                                 Sign, bias=0.0)
            nc.scalar.activation(warm_t[:], nc.const_aps.tensor(0.0, (H1, 1)),
                                 Ident, bias=0.0)

            # ---- constant loads (tiny), then w1t, then per-block xt ----
            b1_t = cpool.tile([H1, 1], F32, tag="b1")
            nc.sync.dma_start(b1_t[:], b1_d)
            w2h_t = cpool.tile([H1, H2], BF16, tag="w2h")
            nc.sync.dma_start(w2h_t[:], w2h_d)
            w2l_t = cpool.tile([H1, H2], BF16, tag="w2l")
            nc.sync.dma_start(w2l_t[:], w2l_d)
            cc_t = cpool.tile([1, NCHUNK * H2], F32, tag="cc160")
            nc.sync.dma_start(cc_t[:], cc_d)
            ones_t = cpool.tile([1, H1], F32, tag="ones")
            nc.vector.memset(ones_t[:], 1.0)
            neg1_t = cpool.tile([H1, 1], F32, tag="neg1")
            nc.vector.memset(neg1_t[:], -1.0)
            w1t_all = cpool.tile([KT, NKT * H1], F32, tag="w1t")
            nc.sync.dma_start(w1t_all[:], w1t_d)
            w1t_tiles = [w1t_all[:, k * H1:(k + 1) * H1] for k in range(NKT)]

            # xt tiles [112, 2048] per K-tile; DMA per (block, k), chained
            # 2-deep so packets arrive sequentially rather than round-robin.
            xt_tiles = [xpool.tile([KT, B], F32, tag=f"xt{k}", name=f"xt{k}")
                        for k in range(NKT)]
            chain = []
            for (bn, c0, w) in BLOCKS:
                for k in range(NKT):
                    dq = nc.sync.dma_start(xt_tiles[k][:, c0:c0 + w],
                                           xt_d[k][:, c0:c0 + w])
                    if len(chain) >= 2:
                        add_dep_helper(dq.ins, chain[-2].ins, sync=True,
                                       reason="stage xt arrival")
                    chain.append(dq)

            # ---- cur1 per block: 7 fp32 K-matmuls -> psum -> sbuf(+b1) ----
            cur1_t = cpool.tile([H1, B], F32, tag="cur1")
            copy_jobs = {}   # block name -> list of (psum_ap, cur1_slice)
            for (bn, c0, w) in BLOCKS:
                jobs = []
                off = c0
                rem = w
                while rem > 0:
                    pw = min(rem, 512)
                    ps = pspool.tile([128, 512], F32, tag="ps",
                                     name=f"cur1_{bn}_{off}")[:, :pw]
                    for k in range(NKT):
                        nc.tensor.matmul(ps, w1t_tiles[k],
                                         xt_tiles[k][:, off:off + pw],
                                         start=(k == 0), stop=(k == NKT - 1))
                    jobs.append((ps, cur1_t[:, off:off + pw]))
                    off += pw
                    rem -= pw
                copy_jobs[bn] = jobs

            def emit_copy(bn):
                for ps, dst in copy_jobs[bn]:
                    # psum -> sbuf with +b1 per-partition bias on ScalarE
                    nc.scalar.activation(dst, ps, Ident, bias=b1_t[:])

            emit_copy("A")
            copy_wave = dict(COPY_WAVE)

            # ---- state ----
            blk = {bn: (c0, w) for (bn, c0, w) in BLOCKS}
            m1_cur = {bn: cur1_t[:, c0:c0 + w] for (bn, c0, w) in BLOCKS}
            m2_cur = None       # layer-2 membrane state [128, 160]
            sg = {}             # (t, bn) -> sign tile (bf16)
            ps2 = {}            # s -> psum slice [128, 160]

            def dve_lif2(s):
                nonlocal m2_cur
                p = ps2.pop(s)
                m2 = m2pool.tile([128, NCHUNK * H2], F32, tag="m2")
                if s == 0:
                    # m2(0) = cur2(0): beta*0 + cur2 - 0
                    nc.vector.tensor_scalar(m2[:], p, 0.0, None, Alu.add)
                else:
                    nc.vector._custom_dve(LIF, out=m2[:], in0=m2_cur[:],
                                          in1=p, s0=beta2, s1=THRESH)
                m2_cur = m2
                nc.sync.dma_start(out_mem[s], m2[:])

            def chunk_mms(t, bn):
                c0, w = blk[bn]
                sgt = sg.pop((t, bn))
                p = ps2[t]
                for c in range(c0 // 128, (c0 + w) // 128):
                    o = p[:, c * H2:(c + 1) * H2]
                    sgc = sgt[:, (c - c0 // 128) * 128:
                              (c - c0 // 128 + 1) * 128]
                    nc.tensor.matmul(o, sgc, w2h_t[:], start=False, stop=False)
                    nc.tensor.matmul(o, sgc, w2l_t[:], start=False,
                                     stop=(bn == "C" and
                                           c == (c0 + w) // 128 - 1))

            # ---- wave loop ----
            for wv in range(N_WAVES):
                # Scalar: pending cur1 copies first
                for bn, cw in list(copy_wave.items()):
                    if cw == wv:
                        emit_copy(bn)
                        del copy_wave[bn]
                # Vector (DVE): layer-1 LIF per block (t=0 is free: m1=cur1)
                for (bn, c0, w) in BLOCKS:
                    t = wv - DELAY[bn]
                    if 1 <= t < STEPS:
                        m1n = m1pool.tile([H1, w], F32, tag=f"m1{bn}")
                        nc.vector._custom_dve(
                            LIF, out=m1n[:], in0=m1_cur[bn][:],
                            in1=cur1_t[:, c0:c0 + w], s0=beta1, s1=THRESH)
                        m1_cur[bn] = m1n
                # DVE: lagged layer-2 LIF (reads PSUM directly)
                s = wv - LAG
                if 0 <= s < STEPS:
                    dve_lif2(s)
                # Scalar: spikes as signs of the just-updated membranes
                for (bn, c0, w) in BLOCKS:
                    t = wv - DELAY[bn]
                    if 0 <= t < STEPS:
                        sgt = sgpool.tile([128, w], BF16, tag=f"sg{bn}")
                        nc.scalar.activation(sgt[:], m1_cur[bn][:], Sign,
                                             bias=neg1_t[:])
                        sg[(t, bn)] = sgt
                # Tensor: chunk matmuls oldest step first, then cc(wv)
                for bn in ("C", "B", "A"):
                    t = wv - CHUNK_WAVE[bn]
                    if 0 <= t < STEPS:
                        chunk_mms(t, bn)
                if wv < STEPS:
                    p = pspool.tile([128, 512], F32, tag="ps",
                                    name=f"ps2_{wv}")[:, :NCHUNK * H2]
                    nc.tensor.matmul(p, ones_t[:], cc_t[:], start=True,
                                     stop=False)
                    ps2[wv] = p

    nc.compile()
    _GRAPH_CACHE[key] = nc
    return nc


def prepare_in_maps(x, W1, b1, W2, b2):
    x = np.asarray(x, dtype=np.float32)
    W1 = np.asarray(W1, dtype=np.float32)
    b1 = np.asarray(b1, dtype=np.float32)
    W2 = np.asarray(W2, dtype=np.float32)
    b2 = np.asarray(b2, dtype=np.float32)
    xf = x.reshape(B_FULL, D_IN)
    xT = xf.T.reshape(NKT, KT, B_FULL)                    # [7, 112, 16384]
    W1T = np.ascontiguousarray(
        W1.T.reshape(NKT, KT, H1).transpose(1, 0, 2).reshape(KT, NKT * H1))
    b1c = np.ascontiguousarray(b1.reshape(H1, 1))
    W2T_half = 0.5 * W2.T                                 # [128, 10]
    w2h = W2T_half.astype(ml_dtypes.bfloat16)
    w2l = (W2T_half - w2h.astype(np.float32)).astype(ml_dtypes.bfloat16)
    ccrow = (0.5 * W2.sum(axis=1) + b2).astype(np.float32)
    cc160 = np.ascontiguousarray(
        np.tile(ccrow, NCHUNK).reshape(1, NCHUNK * H2).astype(np.float32))
    in_maps = []
    for i in range(N_CORES):
        shard = np.ascontiguousarray(xT[:, :, i * B:(i + 1) * B])
        in_maps.append({
            "xt": shard, "w1t": W1T, "b1": b1c,
            "w2h": w2h, "w2l": w2l, "cc160": cc160,
        })
    return in_maps


def kernel(x, W1, b1, W2, b2, beta1, beta2):
    bb1 = float(np.clip(np.float32(beta1), 0.0, 1.0))
    bb2 = float(np.clip(np.float32(beta2), 0.0, 1.0))
    in_maps = prepare_in_maps(x, W1, b1, W2, b2)
    nc = _build_graph(bb1, bb2)
    res = run_bass_kernel_spmd(nc, in_maps, list(range(N_CORES)), trace=False)

    mem_parts = []
    for i in range(N_CORES):
        r = res.results[i]
        # [25, 128, 16*10] -> [25, 2048, 10]; batch = chunk*128 + partition
        mem = r["out_mem"].reshape(STEPS, 128, NCHUNK, H2)
        mem_parts.append(np.transpose(mem, (0, 2, 1, 3)).reshape(STEPS, B, H2))
    mem2 = np.ascontiguousarray(
        np.concatenate(mem_parts, axis=1).astype(np.float32))
    # spikes are a pure function of the (bit-exact) membrane values
    spk2 = (mem2 > np.float32(THRESH)).astype(np.float32)
    return spk2, mem2
